# revision 1
# baseline (speedup 1.0000x reference)
# Trainium2 Bass kernel for CausalSelfAttention (B=2, T=2048, C=1024, H=16, D=64)
# with periodic mask: causal AND (key_col % 4 != 3).
#
# Sharding (8 NeuronCores): core c = (b, g) with b = c//4 (batch), g = c%4
# (head group of 4 heads). Each core computes QKV for its 4 heads, attention,
# and a partial output projection y_heads @ Wp[rows]. Host sums the 4 partials
# per batch and adds bp (tensor-parallel reduce).
#
# Key device-side choices:
#  - All DRAM inputs are pre-arranged on the host into the exact SBUF layout
#    (partition-major), so every bulk DMA is a contiguous per-partition
#    stream at full HBM bandwidth. Window 0 of x is split across rings so
#    the first QKV chain starts ~2us in.
#  - The periodic mask is exploited as compaction: keys at t%4==3 are never
#    attended; K^T is only computed at kept positions (moving-AP compaction
#    in the K projection matmuls) and V rows are gathered by 0/1 selection
#    matmuls.
#  - Scores are produced transposed (S^T[tk_kept, tq]) so softmax-normalized
#    probabilities feed the AV matmul directly as the moving operand.
#  - Softmax row sums come from a 64-wide all-ones block in the V tiles; the
#    reciprocal runs on DVE (reciprocal_approx_fast), keeping the Act engine
#    free for the score exponentials (its true workload).
#  - Emission interleave: QKV chains of window j+1 and output-projection
#    pieces of window j-1 are woven between attention tiles of window j so
#    the PE keeps busy while the Act engine works through the exps.
#  - Output projection stores DMA straight from PSUM (no staging copy).

import ml_dtypes
import numpy as np

B, T, C, H, D = 2, 2048, 1024, 16, 64
HG = 4          # heads per core
CG = HG * D     # = 256 columns of C per core
TK = (T // 4) * 3   # 1536 kept key positions
NTK = TK // 128     # 12 kept-key chunks of 128
SCALE = 1.0 / 8.0   # 1/sqrt(D)

_CACHE = {}


def _split_multi_waits(nc, mybir):
    # The pinned walrus here encodes at most 1 sync-wait per instruction
    # (2 for EventSemaphore). Hoist excess waits onto standalone NoOps that
    # precede the instruction on the same engine.
    f = nc.m.functions[0]
    n = 0
    for b in f.blocks:
        insts = list(b.instructions)
        out = []
        changed = False
        for inst in insts:
            si = inst.sync_info
            if si is not None:
                waits = list(si.on_wait)
                cap = 2 if isinstance(inst, mybir.InstEventSemaphore) else 1
                if len(waits) > cap:
                    for w in waits[cap:]:
                        out.append(mybir.InstNoOp(
                            name=f"{inst.name}-ws{n}", engine=inst.engine,
                            ins=[], outs=[],
                            sync_info=mybir.SyncInfo(on_wait=[w], on_update=[])))
                        n += 1
                    inst.sync_info = mybir.SyncInfo(
                        on_wait=waits[:cap], on_update=list(si.on_update))
                    changed = True
            out.append(inst)
        if changed:
            b.instructions = out
    return n


def _build_bass(split=True):
    import concourse.bass as bass
    import concourse.tile as tile
    import concourse.mybir as mybir

    f32 = mybir.dt.float32
    bf16 = mybir.dt.bfloat16

    nc = bass.Bass("TRN2", debug=False, num_devices=8)

    # host-prearranged, partition-major
    xt_d = nc.dram_tensor("xt", [4, 128, 8, 512], bf16, kind="ExternalInput").ap()
    wq_d = nc.dram_tensor("wq", [128, 2, 8, 128], bf16, kind="ExternalInput").ap()
    wk_d = nc.dram_tensor("wk", [128, 2, 8, 128], bf16, kind="ExternalInput").ap()
    wv_d = nc.dram_tensor("wv", [128, 8, CG], bf16, kind="ExternalInput").ap()
    wp_d = nc.dram_tensor("wp", [128, 2, C], bf16, kind="ExternalInput").ap()
    bq_d = nc.dram_tensor("bq2", [128, 2], f32, kind="ExternalInput").ap()
    bk_d = nc.dram_tensor("bk2", [128, 2], f32, kind="ExternalInput").ap()
    bvb_d = nc.dram_tensor("bvb", [128, HG, D], f32, kind="ExternalInput").ap()
    cm_d = nc.dram_tensor("cmask", [128, 3, 512], bf16, kind="ExternalInput").ap()
    gs_d = nc.dram_tensor("gsel", [128, 6, 128], bf16, kind="ExternalInput").ap()
    out_d = nc.dram_tensor("out", [T, C], bf16, kind="ExternalOutput").ap()

    Ident = mybir.ActivationFunctionType.Identity
    Exp = mybir.ActivationFunctionType.Exp
    MULT = mybir.AluOpType.mult

    with tile.TileContext(nc) as tc, \
         tc.tile_pool(name="persist", bufs=1) as persist, \
         tc.tile_pool(name="work", bufs=1) as work, \
         tc.tile_pool(name="ps_a", space="PSUM", bufs=2) as ps_a, \
         tc.tile_pool(name="ps_s", space="PSUM", bufs=2) as ps_s, \
         tc.tile_pool(name="ps_y", space="PSUM", bufs=2) as ps_y:
        # ---------- persistent SBUF ----------
        qt = [persist.tile([128, T], bf16, name=f"qt{m}", tag=f"qt{m}") for m in range(2)]
        kt = [persist.tile([128, TK], bf16, name=f"kt{m}", tag=f"kt{m}") for m in range(2)]
        vsb = persist.tile([128, NTK, HG, 2 * D], bf16, name="vsb", tag="vsb")
        yt = [persist.tile([128, T], bf16, name=f"yt{m}", tag=f"yt{m}") for m in range(2)]
        cmask = persist.tile([128, 3, 512], bf16, name="cmask", tag="cmask")
        bqs = persist.tile([128, 2], f32, name="bqs", tag="bqs")
        bks = persist.tile([128, 2], f32, name="bks", tag="bks")
        bvb = persist.tile([128, HG, D], f32, name="bvb", tag="bvb")
        bvf = bvb[:].rearrange("p h d -> p (h d)")
        wp_t = persist.tile([128, 2, C], bf16, name="wp_t", tag="wp_t")
        gsel = persist.tile([128, 6, 128], bf16, name="gsel", tag="gsel")
        wq_t = persist.tile([128, 2, 8, 128], bf16, name="wq_t", tag="wq_t")
        wk_t = persist.tile([128, 2, 8, 128], bf16, name="wk_t", tag="wk_t")
        wv_t = persist.tile([128, 8, CG], bf16, name="wv_t", tag="wv_t")

        # ones block for the AV row sums: generated on-chip
        nc.vector.memset(vsb[:, :, :, D:2 * D], 1.0)

        # SWDGE ring: small constants (keeps HWDGE rings for bulk)
        nc.gpsimd.dma_start(bqs[:], bq_d[:])
        nc.gpsimd.dma_start(bks[:], bk_d[:])
        nc.gpsimd.dma_start(bvb[:], bvb_d[:])
        nc.gpsimd.dma_start(cmask[:], cm_d[:])
        nc.gpsimd.dma_start(gsel[:], gs_d[:])

        # HBM is heavily contended (8 cores start identical DMA streams at
        # once): stream the head-critical bytes in fine grains across both
        # HWDGE rings, interleaved in first-use order.
        xt = []
        for j in range(4):
            xw = work.tile([128, 8, 512], bf16, name=f"x{j}", tag=f"xtw{j % 2}",
                           bufs=2)
            xt.append(xw)
        nc.scalar.dma_start(wq_t[:, 0], wq_d[:, 0])
        for k in range(4):
            nc.sync.dma_start(xt[0][:, 2 * k:2 * k + 1, :],
                              xt_d[0, :, 2 * k:2 * k + 1, :])
            nc.scalar.dma_start(xt[0][:, 2 * k + 1:2 * k + 2, :],
                                xt_d[0, :, 2 * k + 1:2 * k + 2, :])
        nc.sync.dma_start(wv_t[:], wv_d[:])
        nc.scalar.dma_start(wk_t[:, 0], wk_d[:, 0])
        nc.sync.dma_start(wq_t[:, 1], wq_d[:, 1])
        nc.scalar.dma_start(wk_t[:, 1], wk_d[:, 1])
        nc.sync.dma_start(xt[1][:, 0:4, :], xt_d[1, :, 0:4, :])
        nc.scalar.dma_start(xt[1][:, 4:8, :], xt_d[1, :, 4:8, :])
        nc.scalar.dma_start(wp_t[:], wp_d[:])
        nc.sync.dma_start(xt[2][:, 0:4, :], xt_d[2, :, 0:4, :])
        nc.scalar.dma_start(xt[2][:, 4:8, :], xt_d[2, :, 4:8, :])
        nc.sync.dma_start(xt[3][:, 0:4, :], xt_d[3, :, 0:4, :])
        nc.scalar.dma_start(xt[3][:, 4:8, :], xt_d[3, :, 4:8, :])

        # kept-token moving view of an x window: [128, k, 128, 0:3]
        def xkept(j):
            return xt[j][:].rearrange("p k (a b) -> p k a b", b=4)[:, :, :, 0:3]

        # ---------- deferred-emission helpers (PE filler work) ----------
        def emit_q(j, m, pool=None):
            pq = (pool or ps_a).tile([128, 512], f32, tag="acc")
            for k in range(8):
                nc.tensor.matmul(pq[:], wq_t[:, m, k, :],
                                 xt[j][:, k, :], start=(k == 0), stop=(k == 7))
            nc.vector.tensor_scalar_add(qt[m][:, 512 * j:512 * (j + 1)],
                                        pq[:], bqs[:, m:m + 1])

        def emit_k(j, m, pool=None):
            # dense matmul (strided moving APs run at half SBUF rate);
            # key compaction happens in the PSUM->SBUF move instead
            pk = (pool or ps_a).tile([128, 512], f32, tag="acc")
            for k in range(8):
                nc.tensor.matmul(pk[:], wk_t[:, m, k, :],
                                 xt[j][:, k, :], start=(k == 0), stop=(k == 7))
            pkc = pk[:].rearrange("p (a b) -> p a b", b=4)[:, :, 0:3]
            nc.vector.tensor_scalar_add(kt[m][:, 384 * j:384 * (j + 1)],
                                        pkc, bks[:, m:m + 1])

        def emit_v(j, mm, vfull, pool=None):
            pv = (pool or ps_a).tile([128, 512], f32, tag="acc")
            for k in range(8):
                nc.tensor.matmul(pv[:, 0:CG],
                                 xt[j][:, k, 128 * mm:128 * (mm + 1)],
                                 wv_t[:, k, :], start=(k == 0), stop=(k == 7))
            vf = work.tile([128, CG], bf16, name=f"vf{j}_{mm}", tag="vf",
                           bufs=6)
            nc.vector.scalar_tensor_tensor(
                out=vf[:], in0=pv[:, 0:CG], scalar=1.0, in1=bvf[:],
                op0=mybir.AluOpType.bypass, op1=mybir.AluOpType.add)
            vfull[mm] = vf

        def emit_gather(j, s, vfull, pool=None):
            i = 3 * j + s
            pvk = (pool or ps_a).tile([128, 512], f32, tag="acc")
            nc.tensor.matmul(pvk[:, 0:CG], gsel[:, 2 * s, :], vfull[s][:],
                             start=True, stop=False)
            nc.tensor.matmul(pvk[:, 0:CG], gsel[:, 2 * s + 1, :], vfull[s + 1][:],
                             start=False, stop=True)
            nc.vector.tensor_copy(
                vsb[:, i, :, 0:D],
                pvk[:, 0:CG].rearrange("p (h d) -> p h d", d=D))

        def emit_qkv_items(j, pools=(None,)):
            vfull = [None] * 4
            fns = [
                lambda p: emit_q(j, 0, p),
                lambda p: emit_k(j, 0, p),
                lambda p: emit_v(j, 0, vfull, p),
                lambda p: emit_v(j, 1, vfull, p),
                lambda p: emit_gather(j, 0, vfull, p),
                lambda p: emit_q(j, 1, p),
                lambda p: emit_k(j, 1, p),
                lambda p: emit_v(j, 2, vfull, p),
                lambda p: emit_gather(j, 1, vfull, p),
                lambda p: emit_v(j, 3, vfull, p),
                lambda p: emit_gather(j, 2, vfull, p),
            ]
            return [lambda fn=fn, p=pools[ix % len(pools)]: fn(p)
                    for ix, fn in enumerate(fns)]

        def emit_proj(m):
            # output projection for token chunk m; bf16 staged, bf16 store
            stage = work.tile([128, C], bf16, tag="stage", bufs=2)
            for n in range(2):
                po = ps_a.tile([128, 512], f32, tag="acc")
                for k2 in range(2):
                    nc.tensor.matmul(
                        po[:], yt[k2][:, 128 * m:128 * (m + 1)],
                        wp_t[:, k2, 512 * n:512 * (n + 1)],
                        start=(k2 == 0), stop=(k2 == 1))
                nc.vector.tensor_copy(stage[:, 512 * n:512 * (n + 1)], po[:])
            if m >= 14:
                # final stores are latency-critical: split across both rings
                nc.sync.dma_start(out_d[128 * m:128 * (m + 1), 0:512],
                                  stage[:, 0:512])
                nc.scalar.dma_start(out_d[128 * m:128 * (m + 1), 512:1024],
                                    stage[:, 512:1024])
            elif m % 2:
                ring = nc.gpsimd if m < 12 else nc.scalar
                ring.dma_start(out_d[128 * m:128 * (m + 1), :], stage[:])
            else:
                nc.sync.dma_start(out_d[128 * m:128 * (m + 1), :], stage[:])

        # ---------- main schedule ----------
        pending = []

        def drain_one():
            if pending:
                pending.pop(0)()

        # window 0: emit only what attention(hp0) needs inline; defer the
        # m=1 head pair and later V work into the attention slots (2 filler
        # drains per tile in window 0)
        w0 = emit_qkv_items(0)
        for it in w0[:5]:
            it()
        pending.extend(w0[5:])

        for j in range(4):
            if j == 0:
                pending.extend(emit_qkv_items(1))
            elif j < 3:
                # qkv filler first (needed by next window's attention), then
                # any deferred output-projection pieces
                pending[0:0] = emit_qkv_items(j + 1)

            jwin = slice(512 * j, 512 * (j + 1))
            ntile = 3 * (j + 1)
            nb0 = ntile - 3  # first boundary tile index
            pys = {}

            def emit_avs(hp, i, pt2, avo):
                for q in range(2):
                    nc.tensor.matmul(
                        pys[hp][q][:, avo:512], vsb[:, i, 2 * hp + q, :],
                        pt2[:, q, avo:512],
                        start=(i == 0), stop=(i == ntile - 1))

            def emit_norm(hp):
                # rec = 1/rowsum via Ln+Exp(-x) on Act, y^T into yt on DVE
                for q in range(2):
                    py = pys[hp][q]
                    rec = work.tile([64, 512], f32, tag="rec", bufs=2)
                    lns = work.tile([64, 512], f32, tag="lns", bufs=2)
                    nc.scalar.activation(lns[:], py[64:128, :],
                                         mybir.ActivationFunctionType.Ln)
                    nc.scalar.activation(rec[:], lns[:], Exp, bias=0.0,
                                         scale=-1.0)
                    nc.vector.tensor_tensor(
                        yt[hp][64 * q:64 * q + 64, jwin],
                        py[0:64, :], rec[:], op=MULT)

            # software-pipelined over a flat (hp, i) tile list: QK(t) and the
            # filler overlap exp(t) on Act; AV trails by one tile
            prev = None
            tiles = [(hp, i) for hp in range(2) for i in range(ntile)]
            for hp, i in tiles:
                if i == 0:
                    pys[hp] = [ps_y.tile([128, 512], f32,
                                         name=f"py{j}_{hp}_{q}", tag="pyo")
                               for q in range(2)]
                ps2 = ps_s.tile([128, 2, 512], f32, tag="ps2")
                pt2 = work.tile([128, 2, 512], bf16, tag="pt2", bufs=4)
                u = i - nb0
                # boundary tiles u=1,2: cols [0:off) are fully masked
                off = (0, 128, 320)[u] if u >= 1 else 0
                for q in range(2):  # q: row group (head 2*hp + q)
                    nc.tensor.matmul(
                        ps2[:, q, off:512],
                        kt[hp][64 * q:64 * q + 64, 128 * i:128 * (i + 1)],
                        qt[hp][64 * q:64 * q + 64,
                               512 * j + off:512 * (j + 1)],
                        start=True, stop=True)
                nc.scalar.activation(pt2[:, :, off:512], ps2[:, :, off:512],
                                     Exp, bias=0.0, scale=SCALE)
                if u == 2:  # u=2 AV stays full width: zero skipped cols
                    nc.gpsimd.memset(pt2[:, :, 0:off], 0.0)
                if u >= 0:  # boundary tile: causal mask (both heads)
                    w = (192, 384, 512)[u]
                    for q in range(2):
                        nc.vector.tensor_tensor(
                            pt2[:, q, off:w], pt2[:, q, off:w],
                            cmask[:, u, off:w], op=MULT)
                drain_one()
                if j == 0:
                    drain_one()
                if prev is not None:
                    emit_avs(*prev)
                    if prev[1] == ntile - 1:
                        emit_norm(prev[0])
                prev = (hp, i, pt2, 128 if u == 1 else 0)
            emit_avs(*prev)
            emit_norm(prev[0])
            while pending:
                drain_one()
            # ---- output projection for the finished query window ----
            if j < 3:
                pending.extend(
                    [lambda m=m: emit_proj(m) for m in range(4 * j, 4 * j + 4)])
            else:
                for m in range(4 * j, 4 * j + 4):
                    emit_proj(m)

    if split:
        _split_multi_waits(nc, mybir)
    return nc


def _get_nc():
    if "nc" not in _CACHE:
        _CACHE["nc"] = _build_bass()
    return _CACHE["nc"]


def _host_maps(inputs):
    x = np.asarray(inputs["x"], np.float32)
    Wq = np.asarray(inputs["Wq"], np.float32)
    Wk = np.asarray(inputs["Wk"], np.float32)
    Wv = np.asarray(inputs["Wv"], np.float32)
    Wp = np.asarray(inputs["Wp"], np.float32)
    bq = np.asarray(inputs["bq"], np.float32)
    bk = np.asarray(inputs["bk"], np.float32)
    bv = np.asarray(inputs["bv"], np.float32)

    # causal masks in compacted key coordinates: 3 boundary chunks
    p = np.arange(128)
    f = np.arange(512)
    cm = np.zeros((128, 3, 512), np.float32)
    for u in range(3):
        q = 128 * u + p
        g = (q // 3) * 4 + (q % 3)
        cm[:, u, :] = (f[None, :] >= g[:, None]).astype(np.float32)

    # V row-gather selection matrices: kept chunk i = 3k+s draws rows from
    # original chunks 4k+s and 4k+s+1; G[s][side][p, m] = 1 iff kept row m
    # maps to row p of that original chunk.
    gs = np.zeros((128, 6, 128), np.float32)
    for s in range(3):
        for m in range(128):
            orr = ((128 * s + m) // 3) * 4 + (128 * s + m) % 3
            side = 0 if orr < 128 * (s + 1) else 1
            gs[orr - 128 * (s + side), 2 * s + side, m] = 1.0

    # pre-arrange into exact SBUF layouts (partition-major)
    xts = []
    for b in range(B):
        xtb = x[b].T.reshape(8, 128, 4, 512).transpose(2, 1, 0, 3)
        xts.append(np.ascontiguousarray(xtb).astype(ml_dtypes.bfloat16))
    def wprep(W, sl):
        return np.ascontiguousarray(
            W[:, sl].reshape(8, 128, CG).transpose(1, 0, 2)
        ).astype(ml_dtypes.bfloat16)
    maps = []
    for c in range(8):
        b, g = c // 4, c % 4
        sl = slice(CG * g, CG * (g + 1))
        maps.append({
            "xt": xts[b],
            "wq": np.ascontiguousarray(
                Wq[:, sl].reshape(8, 128, 2, 128).transpose(1, 2, 0, 3)
            ).astype(ml_dtypes.bfloat16),
            "wk": np.ascontiguousarray(
                Wk[:, sl].reshape(8, 128, 2, 128).transpose(1, 2, 0, 3)
            ).astype(ml_dtypes.bfloat16),
            "wv": wprep(Wv, sl),
            "wp": np.ascontiguousarray(
                Wp[sl, :].reshape(2, 128, C).transpose(1, 0, 2)
            ).astype(ml_dtypes.bfloat16),
            "bq2": np.ascontiguousarray(bq[sl].reshape(2, 128).T),
            "bk2": np.ascontiguousarray(bk[sl].reshape(2, 128).T),
            "bvb": np.ascontiguousarray(
                np.broadcast_to(bv[sl].reshape(HG, D), (128, HG, D))),
            "cmask": cm.astype(ml_dtypes.bfloat16),
            "gsel": gs.astype(ml_dtypes.bfloat16),
        })
    return maps


def _combine(results, inputs):
    bp = np.asarray(inputs["bp"], np.float32)
    out = np.zeros((B, T, C), np.float32)
    for c in range(8):
        out[c // 4] += np.asarray(results[c]["out"]).astype(np.float32)
    out += bp[None, None, :]
    return out


def _run(inputs, profile_dir=None, trace_cores=None):
    nc = _get_nc()
    maps = _host_maps(inputs)
    from concourse.bass_utils import run_bass_kernel_spmd
    if profile_dir is not None:
        import types, sys
        from trn_agent_boot.trn_boot import _ntff_profile_via_ctypes
        hook = _ntff_profile_via_ctypes("/opt/axon/libaxon_pjrt.so")
        with hook(profile_dir, trace_cores or [0]):
            res = run_bass_kernel_spmd(nc, maps, core_ids=list(range(8)))
    else:
        res = run_bass_kernel_spmd(nc, maps, core_ids=list(range(8)))
    return _combine(res.results, inputs)


def kernel(**inputs):
    return _run(inputs)

